# revision 27
# baseline (speedup 1.0000x reference)
"""Trainium2 Bass kernel for a Bayesian (variational) single-layer LSTM.

Reference computation (B=512, S=128, H=512, IN=1, OUT=1):
    W = mu + softplus(rho) * eps            (variational reparameterization)
    u[b,s] = x[b,s] * mask_in[b,s]          (inverted input dropout)
    gates(t) = u[:,t] * W_ih + b + h(t-1) @ W_hh
    i,f,g,o = split(gates); c = f*c + i*g; h = o * tanh(c)
    out = (h(S-1) * mask_out) @ W_lin + b_lin

Strategy: data-parallel over batch (64 rows per core, 8 cores), weights
replicated.  Everything on-chip is kept in a feature-major ("transposed")
layout: gates^T / h^T / c^T with gate-features on partitions and batch on
the free dim, so the recurrence needs no transposes at all:

    gates^T[128-feat tile m, batch] = sum_k W_hh[k-chunk, m-tile].T @ h^T[k-chunk]
                                      (+ [W_ih; b].T @ [u_s; 1])

W_hh tiles are the PE stationary operand in fp16 (fast-weight-load),
h^T is the small moving operand (N=64).  PSUM (fp32) accumulates; the
first gx matmul of each bank carries start=True (clears the bank's
has_written bits) and explicit dep edges force it to issue before the
bank's other matmuls; everything else accumulates order-independently.

The elementwise chain is split into two 128-feature halves so the first
half of h^T (contraction chunks 0,1) is ready while the PE still has
chunk-2/3 work, hiding the recurrence dependency bubble.

Precision: W/h/u in fp16, PSUM + sigmoids/tanh/c in fp32. End-to-end
rel err vs the fp32 reference ~8e-3 (validated against jax on CPU).
"""

import os
import sys

import numpy as np

for _p in ("/opt/trn_rl_repo",):
    if _p not in sys.path:
        sys.path.insert(0, _p)

from concourse import bacc, bass, mybir, tile  # noqa: E402
from concourse.bass_utils import run_bass_kernel_spmd  # noqa: E402
from concourse.tile_rust import add_dep_helper  # noqa: E402

N_CORES = 8
B, S, H, OUT = 512, 128, 512, 1
BL = B // N_CORES            # 64 batch rows per core
G4 = 4 * H                   # 2048 gate features
KC = H // 128                # 4 contraction chunks
MT = G4 // 128               # 16 gate-feature tiles
F32 = mybir.dt.float32
F16 = mybir.dt.float16
AF = mybir.ActivationFunctionType
OP = mybir.AluOpType

_cache = {}


def _softplus_sample(nc, pool, mu_ap, rho_ap, eps_ap, out_f16, rows, tag,
                     scale_ap=None):
    """out_f16[rows, G4] = fp16((mu + softplus(rho) * eps) * scale)."""
    mu = pool.tile([rows, G4], F32, tag="smp_mu", name=f"{tag}_mu")
    rho = pool.tile([rows, G4], F32, tag="smp_rho", name=f"{tag}_rho")
    eps = pool.tile([rows, G4], F32, tag="smp_eps", name=f"{tag}_eps")
    nc.sync.dma_start(mu[:, :], mu_ap)
    nc.sync.dma_start(rho[:, :], rho_ap)
    nc.sync.dma_start(eps[:, :], eps_ap)
    ex = pool.tile([rows, G4], F32, tag="smp_ex", name=f"{tag}_ex")
    nc.scalar.activation(ex[:, :], rho[:, :], AF.Exp)
    sp = pool.tile([rows, G4], F32, tag="smp_sp", name=f"{tag}_sp")
    nc.scalar.activation(sp[:, :], ex[:, :], AF.Ln, bias=1.0)
    t = pool.tile([rows, G4], F32, tag="smp_t", name=f"{tag}_t")
    nc.vector.tensor_mul(t[:, :], sp[:, :], eps[:, :])
    if scale_ap is None:
        nc.vector.tensor_add(out_f16, t[:, :], mu[:, :])
    else:
        w32 = pool.tile([rows, G4], F32, tag="smp_ex", name=f"{tag}_w32")
        nc.vector.tensor_add(w32[:, :], t[:, :], mu[:, :])
        nc.vector.tensor_mul(out_f16, w32[:, :], scale_ap)


def _build():
    if "built" in _cache:
        return _cache["built"]

    dbg = bool(int(os.environ.get("KERNEL_DEBUG", "0")))
    nc = bacc.Bacc(
        "TRN2", target_bir_lowering=False, debug=False, num_devices=N_CORES
    )

    # ---- I/O ----
    def din(name, shape):
        return nc.dram_tensor(name, shape, F32, kind="ExternalInput").ap()

    x_sl = din("x_sl", [BL, S])
    mk_sl = din("mk_sl", [BL, S])
    mo_sl = din("mo_sl", [BL, H])
    wih_mu, wih_rho, eps_ih = din("wih_mu", [1, G4]), din("wih_rho", [1, G4]), din("eps_ih", [1, G4])
    b_mu, b_rho, eps_b = din("b_mu", [1, G4]), din("b_rho", [1, G4]), din("eps_b", [1, G4])
    whh_mu, whh_rho, eps_hh = din("whh_mu", [H, G4]), din("whh_rho", [H, G4]), din("eps_hh", [H, G4])
    wlin = din("wlin", [H, OUT])
    blin = din("blin", [1, OUT])
    out_d = nc.dram_tensor("out", [BL, OUT], F32, kind="ExternalOutput").ap()
    u_scr = nc.dram_tensor("u_scr", [S, BL], F16, kind="Internal").ap()
    dbg_d = {}
    if dbg:
        for nm, shp, dt in [
            ("dbg_u2", [2, S * BL], F16),
            ("dbg_wg", [2, G4], F16),
            ("dbg_w16_0", [128, G4], F16),
            ("dbg_hlast", [128, 256], F16),
        ]:
            dbg_d[nm] = nc.dram_tensor(nm, shp, dt, kind="ExternalOutput").ap()

    # k2/k3 phase: whole gate groups in consumption order g, i, f, o
    K_ORDER = [8, 9, 10, 11, 0, 1, 2, 3, 4, 5, 6, 7, 12, 13, 14, 15]

    with tile.TileContext(nc) as tc:
        with tc.tile_pool(name="const", bufs=1) as const:
            w16 = [
                const.tile([128, G4], F16, tag=f"w16_{k}", name=f"w16_{k}")
                for k in range(KC)
            ]
            wg = const.tile([2, G4], F16, tag="wg", name="wg")
            u2 = const.tile([2, S * BL], F16, tag="u2", name="u2")
            mot = const.tile([128, KC * BL], F16, tag="mot", name="mot")
            wl16 = const.tile([128, KC], F16, tag="wl16", name="wl16")
            bl32 = const.tile([1, OUT], F32, tag="bl32", name="bl32")

            # ------------- prologue (input path first, then weights) -------
            with tc.tile_pool(name="pre", bufs=2) as pre:
                # u = x * mask_in -> transposed + flattened U2 [2, S*BL]
                xt = pre.tile([BL, S], F32, tag="xt", name="xt")
                mkt = pre.tile([BL, S], F32, tag="mkt", name="mkt")
                nc.sync.dma_start(xt[:, :], x_sl)
                nc.sync.dma_start(mkt[:, :], mk_sl)
                u16 = pre.tile([BL, S], F16, tag="u16", name="u16")
                nc.vector.tensor_mul(u16[:, :], xt[:, :], mkt[:, :])
                ut = pre.tile([S, BL], F16, tag="ut", name="ut")
                nc.sync.dma_start_transpose(ut[:, :], u16[:, :])
                nc.sync.dma_start(u_scr, ut[:, :])
                nc.sync.dma_start(
                    u2[0:1, :], u_scr.rearrange("s b -> (s b)")[None, :]
                )
                ones_row = pre.tile([1, S * BL], F16, tag="ones_row", name="ones_row")
                nc.gpsimd.memset(ones_row[:, :], 1.0)
                nc.sync.dma_start(u2[1:2, :], ones_row[:, :])

                # scale patterns: tanh(x)=2*sigmoid(2x)-1 trick needs the
                # g-gate pre-activations doubled; storing h/2 needs all
                # W_hh columns doubled (and W_lin doubled at the output).
                sc_row = pre.tile([1, G4], F32, tag="sc_row", name="sc_row", bufs=1)
                nc.gpsimd.memset(sc_row[:, :], 1.0)
                nc.gpsimd.memset(sc_row[:, 1024:1536], 2.0)
                sc_w = pre.tile([128, G4], F32, tag="sc_w", name="sc_w", bufs=1)
                nc.gpsimd.memset(sc_w[:, :], 2.0)
                nc.gpsimd.memset(sc_w[:, 1024:1536], 4.0)

                # [W_ih; b] fp16 rows -> Wg [2, G4]
                w4row = pre.tile([1, G4], F16, tag="wrow", name="w4row")
                _softplus_sample(nc, pre, wih_mu, wih_rho, eps_ih, w4row[:, :], 1,
                                 "wih", scale_ap=sc_row[:, :])
                brow = pre.tile([1, G4], F16, tag="wrow", name="brow")
                _softplus_sample(nc, pre, b_mu, b_rho, eps_b, brow[:, :], 1,
                                 "bb", scale_ap=sc_row[:, :])
                nc.sync.dma_start(wg[0:1, :], w4row[:, :])
                nc.sync.dma_start(wg[1:2, :], brow[:, :])

                # mask_out^T fp16, W_lin fp16, b_lin
                mo32 = pre.tile([BL, H], F32, tag="mo32", name="mo32")
                nc.sync.dma_start(mo32[:, :], mo_sl)
                mo16 = pre.tile([BL, H], F16, tag="mo16", name="mo16")
                nc.gpsimd.tensor_copy(mo16[:, :], mo32[:, :])
                for k in range(KC):
                    nc.sync.dma_start_transpose(
                        mot[:, BL * k:BL * (k + 1)], mo16[:, 128 * k:128 * (k + 1)]
                    )
                wl32 = pre.tile([128, KC], F32, tag="wl32", name="wl32")
                for k in range(KC):
                    nc.sync.dma_start(
                        wl32[:, k:k + 1], wlin[128 * k:128 * (k + 1), :]
                    )
                nc.gpsimd.tensor_scalar_mul(wl16[:, :], wl32[:, :], 2.0)
                nc.sync.dma_start(bl32[:, :], blin)

                # W_hh sampled to fp16 (bulk DMA; after the small stuff)
                for k in range(KC):
                    _softplus_sample(
                        nc, pre,
                        whh_mu[128 * k:128 * (k + 1), :],
                        whh_rho[128 * k:128 * (k + 1), :],
                        eps_hh[128 * k:128 * (k + 1), :],
                        w16[k][:, :], 128, f"whh{k}", scale_ap=sc_w[:, :],
                    )

            # ------------- recurrence -------------
            with tc.tile_pool(name="work", bufs=4) as work:
              with tc.tile_pool(name="psum", bufs=4, space="PSUM") as psum:
                h_prev = None
                c_prev = None
                for s in range(S):
                    psA = psum.tile([128, 512], F32, tag="psA", name=f"psA_{s}")
                    psB = psum.tile([128, 512], F32, tag="psB", name=f"psB_{s}")
                    banks = (psA, psB)
                    u_s = u2[:, BL * s:BL * (s + 1)]

                    def gx_mm(m, start):
                        pb = banks[m // 8]
                        col = 64 * (m % 8)
                        return nc.tensor.matmul(
                            pb[:, col:col + 64],
                            wg[:, 128 * m:128 * (m + 1)],
                            u_s,
                            start=start, stop=False, skip_group_check=True,
                        )

                    openA = gx_mm(0, True)
                    openB = gx_mm(8, True)
                    for m in list(range(1, 8)) + list(range(9, 16)):
                        r = gx_mm(m, False)
                        opener = openA if m < 8 else openB
                        add_dep_helper(
                            r.ins, opener.ins, reason="bank start=True first"
                        )
                    if h_prev is not None:
                        def k_mm(k, m):
                            pb = banks[m // 8]
                            col = 64 * (m % 8)
                            nc.tensor.matmul(
                                pb[:, col:col + 64],
                                w16[k][:, 128 * m:128 * (m + 1)],
                                h_prev[:, 64 * k:64 * (k + 1)],
                                start=False, stop=(k == KC - 1),
                                skip_group_check=True,
                            )
                        # k0/k1 need only h-half0; then per-slice k2+k3 so
                        # the sigma inputs (half-0 feature tiles) finish early
                        for k in (0, 1):
                            for m in K_ORDER:
                                k_mm(k, m)
                        for m in K_ORDER:
                            k_mm(2, m)
                            k_mm(3, m)
                    # elementwise, sigmoid-only (tanh(x) = 2*sig(2x)-1 with the
                    # doubling pre-folded into the weights).  The cell state
                    # is stored as C = c/2:  C = (sig(2g)-.5)*i_t + f_t*C_prev,
                    # h/2 = (sig(4C)-.5)*o_t.  Per-gate contiguous sigmas.
                    sg = work.tile([128, 256], F32, tag="sg", name=f"sg_{s}")
                    nc.scalar.activation(sg[:, :], psB[:, 0:256], AF.Sigmoid)
                    si = work.tile([128, 256], F32, tag="si", name=f"si_{s}")
                    nc.scalar.activation(si[:, :], psA[:, 0:256], AF.Sigmoid)
                    sf = work.tile([128, 256], F32, tag="sf", name=f"sf_{s}")
                    nc.scalar.activation(sf[:, :], psA[:, 256:512], AF.Sigmoid)
                    so = work.tile([128, 256], F32, tag="so", name=f"so_{s}")
                    nc.scalar.activation(so[:, :], psB[:, 256:512], AF.Sigmoid)
                    t = work.tile([128, 256], F32, tag="tg", name=f"tg_{s}")
                    nc.vector.scalar_tensor_tensor(
                        t[:, :], sg[:, :], 0.5, si[:, :],
                        op0=OP.subtract, op1=OP.mult,
                    )
                    if c_prev is None:
                        c_new = t
                    else:
                        fc = work.tile([128, 256], F32, tag="fc", name=f"fc_{s}")
                        nc.vector.tensor_mul(fc[:, :], sf[:, :], c_prev[:, :])
                        c_new = work.tile([128, 256], F32, tag="cT", name=f"cT_{s}")
                        nc.vector.tensor_add(c_new[:, :], t[:, :], fc[:, :])
                    h_new = work.tile([128, 256], F16, tag="hT", name=f"hT_{s}")
                    for hf in range(2):
                        sl = slice(128 * hf, 128 * (hf + 1))
                        sc = work.tile(
                            [128, 128], F32, tag=f"sc{hf}", name=f"sc{hf}_{s}"
                        )
                        nc.scalar.activation(
                            sc[:, :], c_new[:, sl], AF.Sigmoid, scale=4.0
                        )
                        nc.vector.scalar_tensor_tensor(
                            h_new[:, sl], sc[:, :], 0.5, so[:, sl],
                            op0=OP.subtract, op1=OP.mult,
                        )
                    h_prev, c_prev = h_new, c_new

              # ------------- epilogue (psum pool released; reuse banks) ----
              with tc.tile_pool(name="psum2", bufs=1, space="PSUM") as psum2:
                mh = work.tile([128, KC * BL], F16, tag="mh", name="mh")
                nc.vector.tensor_mul(mh[:, :], h_prev[:, :], mot[:, :])
                pso = psum2.tile([1, BL], F32, tag="pso", name="pso", bufs=1)
                for k in range(KC):
                    nc.tensor.matmul(
                        pso[0:1, :],
                        wl16[:, k:k + 1],
                        mh[:, BL * k:BL * (k + 1)],
                        start=(k == 0), stop=(k == KC - 1),
                    )
                osb = work.tile([1, BL], F32, tag="osb", name="osb")
                nc.vector.tensor_scalar(
                    osb[:, :], pso[0:1, :], bl32[0:1, 0:1], None, op0=OP.add
                )
                nc.sync.dma_start(out_d.rearrange("b o -> o b"), osb[:, :])

                if dbg:
                    nc.sync.dma_start(dbg_d["dbg_u2"], u2[:, :])
                    nc.sync.dma_start(dbg_d["dbg_wg"], wg[:, :])
                    nc.sync.dma_start(dbg_d["dbg_w16_0"], w16[0][:, :])
                    nc.sync.dma_start(dbg_d["dbg_hlast"], h_prev[:, :])

    nc.compile()
    _cache["built"] = nc
    return nc


def kernel(**inputs) -> np.ndarray:
    nc = _build()
    f32 = np.float32

    def c(a):
        return np.ascontiguousarray(np.asarray(a, dtype=f32))

    shared = {
        "wih_mu": c(inputs["W_ih_mu"]).reshape(1, G4),
        "wih_rho": c(inputs["W_ih_rho"]).reshape(1, G4),
        "eps_ih": c(inputs["eps_ih"]).reshape(1, G4),
        "b_mu": c(inputs["b_mu"]).reshape(1, G4),
        "b_rho": c(inputs["b_rho"]).reshape(1, G4),
        "eps_b": c(inputs["eps_b"]).reshape(1, G4),
        "whh_mu": c(inputs["W_hh_mu"]),
        "whh_rho": c(inputs["W_hh_rho"]),
        "eps_hh": c(inputs["eps_hh"]),
        "wlin": c(inputs["W_lin"]).reshape(H, OUT),
        "blin": c(inputs["b_lin"]).reshape(1, OUT),
    }
    x = c(inputs["x"])
    mk = c(inputs["mask_in"]).reshape(B, S)
    mo = c(inputs["mask_out"])
    in_maps = []
    for i in range(N_CORES):
        sl = slice(BL * i, BL * (i + 1))
        m = dict(shared)
        m["x_sl"] = x[sl]
        m["mk_sl"] = mk[sl]
        m["mo_sl"] = mo[sl]
        in_maps.append(m)

    trace = bool(int(os.environ.get("KERNEL_TRACE", "0")))
    trace_cores = None
    if trace and int(os.environ.get("KERNEL_TRACE_ALL", "0")):
        trace_cores = list(range(N_CORES))
    res = run_bass_kernel_spmd(
        nc, in_maps, core_ids=list(range(N_CORES)), trace=trace,
        trace_cores=trace_cores,
    )
    _cache["last_results"] = res
    out = np.concatenate(
        [res.results[i]["out"].reshape(BL, OUT) for i in range(N_CORES)], axis=0
    )
    return out.astype(np.float32)


# revision 28
# speedup vs baseline: 1.1453x; 1.1453x over previous
"""Trainium2 Bass kernel for a Bayesian (variational) single-layer LSTM.

Reference computation (B=512, S=128, H=512, IN=1, OUT=1):
    W = mu + softplus(rho) * eps            (variational reparameterization)
    u[b,s] = x[b,s] * mask_in[b,s]          (inverted input dropout)
    gates(t) = u[:,t] * W_ih + b + h(t-1) @ W_hh
    i,f,g,o = split(gates); c = f*c + i*g; h = o * tanh(c)
    out = (h(S-1) * mask_out) @ W_lin + b_lin

Strategy: data-parallel over batch (64 rows per core, 8 cores), weights
replicated.  Everything on-chip is kept in a feature-major ("transposed")
layout: gates^T / h^T / c^T with gate-features on partitions and batch on
the free dim, so the recurrence needs no transposes at all:

    gates^T[128-feat tile m, batch] = sum_k W_hh[k-chunk, m-tile].T @ h^T[k-chunk]
                                      (+ [W_ih; b].T @ [u_s; 1])

W_hh tiles are the PE stationary operand in fp16 (fast-weight-load),
h^T is the small moving operand (N=64).  PSUM (fp32) accumulates; the
first gx matmul of each bank carries start=True (clears the bank's
has_written bits) and explicit dep edges force it to issue before the
bank's other matmuls; everything else accumulates order-independently.

The elementwise chain is split into two 128-feature halves so the first
half of h^T (contraction chunks 0,1) is ready while the PE still has
chunk-2/3 work, hiding the recurrence dependency bubble.

Precision: W/h/u in fp16, PSUM + sigmoids/tanh/c in fp32. End-to-end
rel err vs the fp32 reference ~8e-3 (validated against jax on CPU).
"""

import os
import sys

import numpy as np

for _p in ("/opt/trn_rl_repo",):
    if _p not in sys.path:
        sys.path.insert(0, _p)

from concourse import bacc, bass, mybir, tile  # noqa: E402
from concourse.bass_utils import run_bass_kernel_spmd  # noqa: E402
from concourse.tile_rust import add_dep_helper  # noqa: E402

N_CORES = 8
B, S, H, OUT = 512, 128, 512, 1
BL = B // N_CORES            # 64 batch rows per core
G4 = 4 * H                   # 2048 gate features
KC = H // 128                # 4 contraction chunks
MT = G4 // 128               # 16 gate-feature tiles
F32 = mybir.dt.float32
F16 = mybir.dt.float16
AF = mybir.ActivationFunctionType
OP = mybir.AluOpType

_cache = {}


def _softplus_sample(nc, pool, mu_ap, rho_ap, eps_ap, out_f16, rows, tag,
                     scale_ap=None):
    """out_f16[rows, G4] = fp16((mu + softplus(rho) * eps) * scale)."""
    mu = pool.tile([rows, G4], F32, tag="smp_mu", name=f"{tag}_mu")
    rho = pool.tile([rows, G4], F32, tag="smp_rho", name=f"{tag}_rho")
    eps = pool.tile([rows, G4], F32, tag="smp_eps", name=f"{tag}_eps")
    nc.sync.dma_start(mu[:, :], mu_ap)
    nc.sync.dma_start(rho[:, :], rho_ap)
    nc.sync.dma_start(eps[:, :], eps_ap)
    ex = pool.tile([rows, G4], F32, tag="smp_ex", name=f"{tag}_ex")
    nc.scalar.activation(ex[:, :], rho[:, :], AF.Exp)
    sp = pool.tile([rows, G4], F32, tag="smp_sp", name=f"{tag}_sp")
    nc.scalar.activation(sp[:, :], ex[:, :], AF.Ln, bias=1.0)
    t = pool.tile([rows, G4], F32, tag="smp_t", name=f"{tag}_t")
    nc.vector.tensor_mul(t[:, :], sp[:, :], eps[:, :])
    if scale_ap is None:
        nc.vector.tensor_add(out_f16, t[:, :], mu[:, :])
    else:
        w32 = pool.tile([rows, G4], F32, tag="smp_ex", name=f"{tag}_w32")
        nc.vector.tensor_add(w32[:, :], t[:, :], mu[:, :])
        nc.vector.tensor_mul(out_f16, w32[:, :], scale_ap)


def _build():
    if "built" in _cache:
        return _cache["built"]

    dbg = bool(int(os.environ.get("KERNEL_DEBUG", "0")))
    nc = bacc.Bacc(
        "TRN2", target_bir_lowering=False, debug=False, num_devices=N_CORES
    )

    # ---- I/O ----
    def din(name, shape):
        return nc.dram_tensor(name, shape, F32, kind="ExternalInput").ap()

    x_sl = din("x_sl", [BL, S])
    mk_sl = din("mk_sl", [BL, S])
    mo_sl = din("mo_sl", [BL, H])
    wih_mu, wih_rho, eps_ih = din("wih_mu", [1, G4]), din("wih_rho", [1, G4]), din("eps_ih", [1, G4])
    b_mu, b_rho, eps_b = din("b_mu", [1, G4]), din("b_rho", [1, G4]), din("eps_b", [1, G4])
    whh_mu, whh_rho, eps_hh = din("whh_mu", [H, G4]), din("whh_rho", [H, G4]), din("eps_hh", [H, G4])
    wlin = din("wlin", [H, OUT])
    blin = din("blin", [1, OUT])
    out_d = nc.dram_tensor("out", [BL, OUT], F32, kind="ExternalOutput").ap()
    u_scr = nc.dram_tensor("u_scr", [S, BL], F16, kind="Internal").ap()
    dbg_d = {}
    if dbg:
        for nm, shp, dt in [
            ("dbg_u2", [2, S * BL], F16),
            ("dbg_wg", [2, G4], F16),
            ("dbg_w16_0", [128, G4], F16),
            ("dbg_hlast", [128, 256], F16),
        ]:
            dbg_d[nm] = nc.dram_tensor(nm, shp, dt, kind="ExternalOutput").ap()

    # k2/k3 phase: whole gate groups in consumption order g, i, f, o
    K_ORDER = [8, 9, 10, 11, 0, 1, 2, 3, 4, 5, 6, 7, 12, 13, 14, 15]

    with tile.TileContext(nc) as tc:
        with tc.tile_pool(name="const", bufs=1) as const:
            w16 = [
                const.tile([128, G4], F16, tag=f"w16_{k}", name=f"w16_{k}")
                for k in range(KC)
            ]
            wg = const.tile([2, G4], F16, tag="wg", name="wg")
            u2 = const.tile([2, S * BL], F16, tag="u2", name="u2")
            mot = const.tile([128, KC * BL], F16, tag="mot", name="mot")
            wl16 = const.tile([128, KC], F16, tag="wl16", name="wl16")
            bl32 = const.tile([1, OUT], F32, tag="bl32", name="bl32")

            # ------------- prologue (input path first, then weights) -------
            with tc.tile_pool(name="pre", bufs=2) as pre:
                # u = x * mask_in -> transposed + flattened U2 [2, S*BL]
                xt = pre.tile([BL, S], F32, tag="xt", name="xt")
                mkt = pre.tile([BL, S], F32, tag="mkt", name="mkt")
                nc.sync.dma_start(xt[:, :], x_sl)
                nc.sync.dma_start(mkt[:, :], mk_sl)
                u16 = pre.tile([BL, S], F16, tag="u16", name="u16")
                nc.vector.tensor_mul(u16[:, :], xt[:, :], mkt[:, :])
                ut = pre.tile([S, BL], F16, tag="ut", name="ut")
                nc.sync.dma_start_transpose(ut[:, :], u16[:, :])
                nc.sync.dma_start(u_scr, ut[:, :])
                nc.sync.dma_start(
                    u2[0:1, :], u_scr.rearrange("s b -> (s b)")[None, :]
                )
                ones_row = pre.tile([1, S * BL], F16, tag="ones_row", name="ones_row")
                nc.gpsimd.memset(ones_row[:, :], 1.0)
                nc.sync.dma_start(u2[1:2, :], ones_row[:, :])

                # scale patterns: tanh(x)=2*sigmoid(2x)-1 trick needs the
                # g-gate pre-activations doubled; storing h/2 needs all
                # W_hh columns doubled (and W_lin doubled at the output).
                sc_row = pre.tile([1, G4], F32, tag="sc_row", name="sc_row", bufs=1)
                nc.gpsimd.memset(sc_row[:, :], 1.0)
                nc.gpsimd.memset(sc_row[:, 1024:1536], 2.0)
                sc_w = pre.tile([128, G4], F32, tag="sc_w", name="sc_w", bufs=1)
                nc.gpsimd.memset(sc_w[:, :], 2.0)
                nc.gpsimd.memset(sc_w[:, 1024:1536], 4.0)

                # [W_ih; b] fp16 rows -> Wg [2, G4]
                w4row = pre.tile([1, G4], F16, tag="wrow", name="w4row")
                _softplus_sample(nc, pre, wih_mu, wih_rho, eps_ih, w4row[:, :], 1,
                                 "wih", scale_ap=sc_row[:, :])
                brow = pre.tile([1, G4], F16, tag="wrow", name="brow")
                _softplus_sample(nc, pre, b_mu, b_rho, eps_b, brow[:, :], 1,
                                 "bb", scale_ap=sc_row[:, :])
                nc.sync.dma_start(wg[0:1, :], w4row[:, :])
                nc.sync.dma_start(wg[1:2, :], brow[:, :])

                # mask_out^T fp16, W_lin fp16, b_lin
                mo32 = pre.tile([BL, H], F32, tag="mo32", name="mo32")
                nc.sync.dma_start(mo32[:, :], mo_sl)
                mo16 = pre.tile([BL, H], F16, tag="mo16", name="mo16")
                nc.gpsimd.tensor_copy(mo16[:, :], mo32[:, :])
                for k in range(KC):
                    nc.sync.dma_start_transpose(
                        mot[:, BL * k:BL * (k + 1)], mo16[:, 128 * k:128 * (k + 1)]
                    )
                wl32 = pre.tile([128, KC], F32, tag="wl32", name="wl32")
                for k in range(KC):
                    nc.sync.dma_start(
                        wl32[:, k:k + 1], wlin[128 * k:128 * (k + 1), :]
                    )
                nc.gpsimd.tensor_scalar_mul(wl16[:, :], wl32[:, :], 2.0)
                nc.sync.dma_start(bl32[:, :], blin)

                # W_hh sampled to fp16 (bulk DMA; after the small stuff)
                for k in range(KC):
                    _softplus_sample(
                        nc, pre,
                        whh_mu[128 * k:128 * (k + 1), :],
                        whh_rho[128 * k:128 * (k + 1), :],
                        eps_hh[128 * k:128 * (k + 1), :],
                        w16[k][:, :], 128, f"whh{k}", scale_ap=sc_w[:, :],
                    )

            # ------------- recurrence -------------
            # One PSUM bank per gate so ACT sigma reads never share a bank
            # with in-flight PE writes (bank-collision serialization).
            with tc.tile_pool(name="work", bufs=4) as work:
              with tc.tile_pool(name="psum", bufs=2, space="PSUM") as psum:
                h_prev = None
                c_prev = None
                # gate order for emission: g first (needed first), then i, f, o
                GATES = (2, 0, 1, 3)   # m-tile group: gate*4 .. gate*4+3
                for s in range(S):
                    pbk = {}
                    for gt in GATES:
                        pbk[gt] = psum.tile(
                            [128, 256], F32, tag=f"ps{gt}", name=f"ps{gt}_{s}"
                        )
                    u_s = u2[:, BL * s:BL * (s + 1)]

                    def gx_mm(m, start):
                        col = 64 * (m % 4)
                        return nc.tensor.matmul(
                            pbk[m // 4][:, col:col + 64],
                            wg[:, 128 * m:128 * (m + 1)],
                            u_s,
                            start=start, stop=False, skip_group_check=True,
                        )

                    for gt in GATES:
                        opener = gx_mm(4 * gt, True)
                        for m in range(4 * gt + 1, 4 * gt + 4):
                            r = gx_mm(m, False)
                            add_dep_helper(
                                r.ins, opener.ins, reason="bank start first"
                            )
                    if h_prev is not None:
                        def k_mm(k, m):
                            col = 64 * (m % 4)
                            nc.tensor.matmul(
                                pbk[m // 4][:, col:col + 64],
                                w16[k][:, 128 * m:128 * (m + 1)],
                                h_prev[:, 64 * k:64 * (k + 1)],
                                start=False, stop=(k == KC - 1),
                                skip_group_check=True,
                            )
                        # k0/k1 need only h-half0; then whole gate groups
                        # (g,i,f,o) finish in sigma-consumption order
                        for k in (0, 1):
                            for gt in GATES:
                                for m in range(4 * gt, 4 * gt + 4):
                                    k_mm(k, m)
                        for gt in GATES:
                            for m in range(4 * gt, 4 * gt + 4):
                                k_mm(2, m)
                                k_mm(3, m)
                    # elementwise, sigmoid-only (tanh(x) = 2*sig(2x)-1 with the
                    # doubling pre-folded into the weights).  The cell state
                    # is stored as C = c/2:  C = (sig(2g)-.5)*i_t + f_t*C_prev,
                    # h/2 = (sig(4C)-.5)*o_t.  One sigma per gate bank.
                    sg = work.tile([128, 256], F32, tag="sg", name=f"sg_{s}")
                    nc.scalar.activation(sg[:, :], pbk[2][:, :], AF.Sigmoid)
                    si = work.tile([128, 256], F32, tag="si", name=f"si_{s}")
                    nc.scalar.activation(si[:, :], pbk[0][:, :], AF.Sigmoid)
                    sf = work.tile([128, 256], F32, tag="sf", name=f"sf_{s}")
                    nc.scalar.activation(sf[:, :], pbk[1][:, :], AF.Sigmoid)
                    so = work.tile([128, 256], F32, tag="so", name=f"so_{s}")
                    nc.scalar.activation(so[:, :], pbk[3][:, :], AF.Sigmoid)
                    t = work.tile([128, 256], F32, tag="tg", name=f"tg_{s}")
                    nc.vector.scalar_tensor_tensor(
                        t[:, :], sg[:, :], 0.5, si[:, :],
                        op0=OP.subtract, op1=OP.mult,
                    )
                    if c_prev is None:
                        c_new = t
                    else:
                        fc = work.tile([128, 256], F32, tag="fc", name=f"fc_{s}")
                        nc.vector.tensor_mul(fc[:, :], sf[:, :], c_prev[:, :])
                        c_new = work.tile([128, 256], F32, tag="cT", name=f"cT_{s}")
                        nc.vector.tensor_add(c_new[:, :], t[:, :], fc[:, :])
                    h_new = work.tile([128, 256], F16, tag="hT", name=f"hT_{s}")
                    for hf in range(2):
                        sl = slice(128 * hf, 128 * (hf + 1))
                        sc = work.tile(
                            [128, 128], F32, tag=f"sc{hf}", name=f"sc{hf}_{s}"
                        )
                        nc.scalar.activation(
                            sc[:, :], c_new[:, sl], AF.Sigmoid, scale=4.0
                        )
                        nc.vector.scalar_tensor_tensor(
                            h_new[:, sl], sc[:, :], 0.5, so[:, sl],
                            op0=OP.subtract, op1=OP.mult,
                        )
                    h_prev, c_prev = h_new, c_new

              # ------------- epilogue (psum pool released; reuse banks) ----
              with tc.tile_pool(name="psum2", bufs=1, space="PSUM") as psum2:
                mh = work.tile([128, KC * BL], F16, tag="mh", name="mh")
                nc.vector.tensor_mul(mh[:, :], h_prev[:, :], mot[:, :])
                pso = psum2.tile([1, BL], F32, tag="pso", name="pso", bufs=1)
                for k in range(KC):
                    nc.tensor.matmul(
                        pso[0:1, :],
                        wl16[:, k:k + 1],
                        mh[:, BL * k:BL * (k + 1)],
                        start=(k == 0), stop=(k == KC - 1),
                    )
                osb = work.tile([1, BL], F32, tag="osb", name="osb")
                nc.vector.tensor_scalar(
                    osb[:, :], pso[0:1, :], bl32[0:1, 0:1], None, op0=OP.add
                )
                nc.sync.dma_start(out_d.rearrange("b o -> o b"), osb[:, :])

                if dbg:
                    nc.sync.dma_start(dbg_d["dbg_u2"], u2[:, :])
                    nc.sync.dma_start(dbg_d["dbg_wg"], wg[:, :])
                    nc.sync.dma_start(dbg_d["dbg_w16_0"], w16[0][:, :])
                    nc.sync.dma_start(dbg_d["dbg_hlast"], h_prev[:, :])

    nc.compile()
    _cache["built"] = nc
    return nc


def kernel(**inputs) -> np.ndarray:
    nc = _build()
    f32 = np.float32

    def c(a):
        return np.ascontiguousarray(np.asarray(a, dtype=f32))

    shared = {
        "wih_mu": c(inputs["W_ih_mu"]).reshape(1, G4),
        "wih_rho": c(inputs["W_ih_rho"]).reshape(1, G4),
        "eps_ih": c(inputs["eps_ih"]).reshape(1, G4),
        "b_mu": c(inputs["b_mu"]).reshape(1, G4),
        "b_rho": c(inputs["b_rho"]).reshape(1, G4),
        "eps_b": c(inputs["eps_b"]).reshape(1, G4),
        "whh_mu": c(inputs["W_hh_mu"]),
        "whh_rho": c(inputs["W_hh_rho"]),
        "eps_hh": c(inputs["eps_hh"]),
        "wlin": c(inputs["W_lin"]).reshape(H, OUT),
        "blin": c(inputs["b_lin"]).reshape(1, OUT),
    }
    x = c(inputs["x"])
    mk = c(inputs["mask_in"]).reshape(B, S)
    mo = c(inputs["mask_out"])
    in_maps = []
    for i in range(N_CORES):
        sl = slice(BL * i, BL * (i + 1))
        m = dict(shared)
        m["x_sl"] = x[sl]
        m["mk_sl"] = mk[sl]
        m["mo_sl"] = mo[sl]
        in_maps.append(m)

    trace = bool(int(os.environ.get("KERNEL_TRACE", "0")))
    trace_cores = None
    if trace and int(os.environ.get("KERNEL_TRACE_ALL", "0")):
        trace_cores = list(range(N_CORES))
    res = run_bass_kernel_spmd(
        nc, in_maps, core_ids=list(range(N_CORES)), trace=trace,
        trace_cores=trace_cores,
    )
    _cache["last_results"] = res
    out = np.concatenate(
        [res.results[i]["out"].reshape(BL, OUT) for i in range(N_CORES)], axis=0
    )
    return out.astype(np.float32)
